# revision 4
# baseline (speedup 1.0000x reference)
"""EnergyHead kernel for Trainium2 (8 NeuronCores, batch-parallel).

Computes, per batch element:
    xH = x @ W_H.T
    scores = x @ xH.T  (strict lower-triangular causal mask, diag excluded)
    wei = softmax(scores); fully-masked row 0 zeroed
    out = -(wei @ xH)

Sharding: data-parallel over B=8 across 8 cores. Each core receives
xT = f32r(x[b].T) [C,T] and wT = f32r(W_H.T) [C,C]; matmul phases 1-2 run in
float32r (11-bit mantissa, full PE speed), phase 3 (attention-weighted sum,
near-one-hot weights) in bf16. End-to-end rel err vs fp32 reference ~2e-3.
"""
import sys
import functools

sys.path.insert(0, "/opt/trn_rl_repo")
import numpy as np

B, T, C = 8, 2048, 1024
NCORES = 8
P = 128                      # partition dim
QT = T // P                  # 16 q-tiles
CCH = C // P                 # 8 contraction chunks
CB = C // 512                # 2 column blocks of the output
NEG_BIG = -1e30


def _f32r_round(a: np.ndarray) -> np.ndarray:
    """Round fp32 to the f32r format (round-to-nearest, 11 mantissa bits)."""
    au = np.ascontiguousarray(a, dtype=np.float32).view(np.uint32)
    return ((au.astype(np.uint64) + 0x800).astype(np.uint32) & np.uint32(0xFFFFF000)).view(np.float32)


def _sblocks(i: int) -> list[int]:
    """Split S=128*(i+1) score columns into matmul N-blocks, avoiding
    blocks <256 (f32r runs 4 cycles/row below 256) except the i=0 case."""
    S = P * (i + 1)
    if S <= 512:
        return [S]
    k4, rem = divmod(S, 512)
    if rem == 0:
        return [512] * k4
    if rem == P:
        return [512] * (k4 - 1) + [384, 256]
    return [512] * k4 + [rem]


@functools.lru_cache(maxsize=1)
def _build():
    import concourse.bacc as bacc
    import concourse.tile as tile
    from concourse import mybir
    from concourse.masks import make_identity

    f32 = mybir.dt.float32
    f32r = mybir.dt.float32r
    bf16 = mybir.dt.bfloat16
    X = mybir.AxisListType.X
    Exp = mybir.ActivationFunctionType.Exp
    Copy = mybir.ActivationFunctionType.Copy

    nc = bacc.Bacc("TRN2", target_bir_lowering=False, debug=False,
                   enable_asserts=False, num_devices=NCORES)

    xT_d = nc.dram_tensor("xT", [C, T], f32r, kind="ExternalInput").ap()
    wT_d = nc.dram_tensor("wT", [C, C], f32r, kind="ExternalInput").ap()
    out_d = nc.dram_tensor("out", [T, C], f32, kind="ExternalOutput").ap()

    with tile.TileContext(nc) as tc:
        with tc.tile_pool(name="pers", bufs=1) as pers, \
             tc.tile_pool(name="wei", bufs=2) as weip, \
             tc.tile_pool(name="weiT", bufs=17) as weitp, \
             tc.tile_pool(name="osb", bufs=2) as outsp, \
             tc.tile_pool(name="stats", bufs=2) as statsp, \
             tc.tile_pool(name="blk", bufs=4, space="PSUM") as ps512, \
             tc.tile_pool(name="outp", bufs=1, space="PSUM") as outpp, \
             tc.tile_pool(name="tp", bufs=2, space="PSUM") as tpp:

            # ---- constants
            ident_f = pers.tile([P, P], f32, tag="ident_f")
            make_identity(nc, ident_f[:])
            ident_b = pers.tile([P, P], bf16, tag="ident_b")
            make_identity(nc, ident_b[:])
            # strict-causal additive mask for the diagonal block:
            # diagmask[p, f] = 0 if f < p else NEG_BIG
            diagmask = pers.tile([P, P], f32, tag="diagmask")
            nc.gpsimd.memset(diagmask[:], 0.0)
            nc.gpsimd.affine_select(
                out=diagmask[:], in_=diagmask[:],
                compare_op=mybir.AluOpType.is_gt,
                fill=NEG_BIG, base=0, pattern=[[-1, P]], channel_multiplier=1,
            )
            # -1 everywhere except row 0 (zeroes the fully-masked first row)
            rowmask0 = pers.tile([P, 1], f32, tag="rowmask0")
            nc.gpsimd.memset(rowmask0[:], -1.0)
            nc.gpsimd.memset(rowmask0[0:1, :], 0.0)

            # ---- persistent arrays
            # xT tiles [c-chunk][t-block]: [128, 512] each, so phase-1 can
            # start as soon as the first pieces arrive
            xT_sb = [[pers.tile([P, 512], f32r, tag=f"xT{c}_{tb}", name=f"xT{c}_{tb}")
                      for tb in range(4)] for c in range(CCH)]
            for c in range(CCH):
                for tb in range(4):
                    nc.sync.dma_start(xT_sb[c][tb][:],
                                      xT_d[P * c:P * (c + 1), 512 * tb:512 * (tb + 1)])
            xHT_sb = [pers.tile([P, T], f32r, tag=f"xHT{d}", name=f"xHT{d}") for d in range(CCH)]
            xH_sb = [pers.tile([P, C], bf16, tag=f"xH{t}", name=f"xH{t}") for t in range(QT)]

            # ---- phase 1a: xHT[d, t] = W_H @ x.T  (f32r)
            # wT loaded in two 512-column halves to halve its SBUF footprint
            with tc.tile_pool(name="wt", bufs=1) as wtp:
                for half in range(2):
                    wTh = [wtp.tile([P, 512], f32r, tag=f"wT{c}",
                                    name=f"wT{c}h{half}") for c in range(CCH)]
                    for c in range(CCH):
                        nc.sync.dma_start(
                            wTh[c][:],
                            wT_d[P * c:P * (c + 1), 512 * half:512 * (half + 1)])
                    for tb in range(4):
                        for dd in range(4):
                            d = 4 * half + dd
                            pmm = ps512.tile([P, 512], f32, tag="blk")
                            for c in range(CCH):
                                nc.tensor.matmul(
                                    pmm[:],
                                    wTh[c][:, P * dd:P * (dd + 1)],
                                    xT_sb[c][tb][:],
                                    start=(c == 0), stop=(c == CCH - 1))
                            nc.vector.tensor_copy(
                                xHT_sb[d][:, 512 * tb:512 * (tb + 1)], pmm[:])

            # ---- q-tile loop (software-pipelined emission)
            def emit_scores(i):
                blks = []
                off = 0
                tb = i // 4
                qs = P * (i % 4)
                for n in _sblocks(i):
                    pmm = ps512.tile([P, 512], f32, tag="blk")
                    for c in range(CCH):
                        nc.tensor.matmul(
                            pmm[:, :n],
                            xT_sb[c][tb][:, qs:qs + P],
                            xHT_sb[c][:, off:off + n],
                            start=(c == 0), stop=(c == CCH - 1))
                    blks.append((pmm, off, n))
                    off += n
                # mask the diagonal block (last 128 columns)
                pl, offl, nl = blks[-1]
                nc.vector.tensor_add(pl[:, nl - P:nl], pl[:, nl - P:nl], diagmask[:])
                return blks

            def emit_phase1b(t):
                # xH natural (bf16) for rows 128t..128t+127 via PE transpose
                for d in range(CCH):
                    tps = tpp.tile([P, P], f32, tag="tp")
                    nc.tensor.transpose(
                        tps[:], xHT_sb[d][:, P * t:P * (t + 1)].bitcast(f32),
                        ident_f[:])
                    nc.vector.tensor_copy(xH_sb[t][:, P * d:P * (d + 1)], tps[:])

            def emit_softmax_out(i, blks):
                nblk = len(blks)
                negblk = statsp.tile([P, 4], f32, tag="negblk")
                for k, (pmm, off, n) in enumerate(blks):
                    nc.vector.reduce_max(negblk[:, k:k + 1], pmm[:, :n],
                                         axis=X, negate=True)
                if nblk > 1:
                    gneg = statsp.tile([P, 1], f32, tag="gneg")
                    nc.vector.tensor_reduce(gneg[:], negblk[:, :nblk],
                                            axis=X, op=mybir.AluOpType.min)
                    gneg_ap = gneg[:]
                else:
                    gneg_ap = negblk[:, 0:1]

                sums = statsp.tile([P, 4], f32, tag="sums")
                nc.gpsimd.memset(sums[:, :nblk], 0.0)
                S = P * (i + 1)
                wei = weip.tile([P, S], bf16, tag="wei")
                for k, (pmm, off, n) in enumerate(blks):
                    nc.scalar.activation(wei[:, off:off + n], pmm[:, :n], Exp,
                                         bias=gneg_ap, scale=1.0,
                                         accum_out=sums[:, k:k + 1])
                if nblk > 1:
                    sumtot = statsp.tile([P, 1], f32, tag="sumtot")
                    nc.vector.reduce_sum(sumtot[:], sums[:, :nblk], axis=X)
                    sumtot_ap = sumtot[:]
                else:
                    sumtot_ap = sums[:, 0:1]
                recip = statsp.tile([P, 1], f32, tag="recip")
                nc.vector.reciprocal(recip[:], sumtot_ap)
                if i == 0:
                    nc.vector.tensor_mul(recip[:], recip[:], rowmask0[:])
                else:
                    nc.vector.tensor_scalar_mul(recip[:], recip[:], -1.0)

                # transpose wei 128-blocks -> weiT (bf16)
                weiTs = []
                for k in range(i + 1):
                    tps = tpp.tile([P, P], bf16, tag="tp")
                    nc.tensor.transpose(tps[:], wei[:, P * k:P * (k + 1)],
                                        ident_b[:])
                    wtt = weitp.tile([P, P], bf16, tag="weiT")
                    nc.vector.tensor_copy(wtt[:], tps[:])
                    weiTs.append(wtt)

                # out = wei @ xH  (bf16, accumulate over s-chunks)
                outp = outpp.tile([P, C], f32, tag="outp")
                for cb in range(CB):
                    for k in range(i + 1):
                        nc.tensor.matmul(
                            outp[:, 512 * cb:512 * (cb + 1)],
                            weiTs[k][:],
                            xH_sb[k][:, 512 * cb:512 * (cb + 1)],
                            start=(k == 0), stop=(k == i))
                osb = outsp.tile([P, C], f32, tag="osb")
                nc.scalar.activation(osb[:], outp[:], Copy, bias=0.0,
                                     scale=recip[:])
                nc.sync.dma_start(out_d[P * i:P * (i + 1), :], osb[:])

            blks = emit_scores(0)
            for i in range(QT):
                nxt = emit_scores(i + 1) if i + 1 < QT else None
                emit_phase1b(i)
                emit_softmax_out(i, blks)
                blks = nxt

    nc.compile()
    return nc


def kernel(x: np.ndarray, W_H: np.ndarray) -> np.ndarray:
    from concourse import bass_utils

    x = np.asarray(x, dtype=np.float32)
    W_H = np.asarray(W_H, dtype=np.float32)
    nc = _build()
    wT = _f32r_round(W_H.T)
    in_maps = [{"xT": _f32r_round(x[b].T), "wT": wT} for b in range(B)]
    res = bass_utils.run_bass_kernel_spmd(nc, in_maps, core_ids=list(range(NCORES)))
    return np.stack([res.results[b]["out"] for b in range(B)])


if __name__ == "__main__":
    x = np.random.randn(B, T, C).astype(np.float32)
    W = (np.random.randn(C, C) / np.sqrt(C)).astype(np.float32)
    out = kernel(x, W)
    print("out", out.shape, out.dtype)


# revision 49
# speedup vs baseline: 386.6328x; 386.6328x over previous
"""EnergyHead kernel for Trainium2 (8 NeuronCores, batch-parallel).

Computes, per batch element:
    xH = x @ W_H.T
    scores = x @ xH.T  (strict lower-triangular causal mask, diag excluded)
    wei = softmax(scores); fully-masked row 0 zeroed
    out = -(wei @ xH)

Sharding: data-parallel over B=8 across 8 cores. Each core receives
xT = f32r(x[b].T) [C,T] and wT = f32r(W_H.T) [C,C]; matmul phases 1-2 run in
float32r (11-bit mantissa, full PE speed), phase 3 (attention-weighted sum,
near-one-hot weights) in bf16. End-to-end rel err vs fp32 reference ~2e-3.
"""
import sys
import os
import functools
from contextlib import ExitStack

sys.path.insert(0, "/opt/trn_rl_repo")
import numpy as np

# experiment flags (read once at import)
CFG_WT = os.environ.get("K_WT", "full")        # quarters | halves_war
CFG_SPLIT_WEIT = int(os.environ.get("K_SPLIT_WEIT", "1"))  # 1 | 2
CFG_SCALE = os.environ.get("K_SCALE", "act")       # act | dve
CFG_DEPTH = int(os.environ.get("K_DEPTH", "2"))    # 1 | 2
CFG_DEBUG = int(os.environ.get("K_DEBUG", "0"))

B, T, C = 8, 2048, 1024
NCORES = 8
P = 128                      # partition dim
QT = T // P                  # 16 q-tiles
CCH = C // P                 # 8 contraction chunks
CB = C // 512                # 2 column blocks of the output
NEG_BIG = -1e30


def _f32r_round(a: np.ndarray) -> np.ndarray:
    """Round fp32 to the f32r format (round-to-nearest, 11 mantissa bits)."""
    au = np.ascontiguousarray(a, dtype=np.float32).view(np.uint32)
    return ((au.astype(np.uint64) + 0x800).astype(np.uint32) & np.uint32(0xFFFFF000)).view(np.float32)


def _sblocks(i: int) -> list[int]:
    """Split S=128*(i+1) score columns into matmul N-blocks, avoiding
    blocks <256 (f32r runs 4 cycles/row below 256) except the i=0 case."""
    S = P * (i + 1)
    if S <= 512:
        return [S]
    k4, rem = divmod(S, 512)
    if rem == 0:
        return [512] * k4
    if rem == P:
        return [512] * (k4 - 1) + [384, 256]
    return [512] * k4 + [rem]


@functools.lru_cache(maxsize=1)
def _build():
    import concourse.bacc as bacc
    import concourse.tile as tile
    from concourse import mybir
    from concourse.masks import make_identity

    f32 = mybir.dt.float32
    f32r = mybir.dt.float32r
    bf16 = mybir.dt.bfloat16
    X = mybir.AxisListType.X
    Exp = mybir.ActivationFunctionType.Exp
    Copy = mybir.ActivationFunctionType.Copy

    nc = bacc.Bacc("TRN2", target_bir_lowering=False, debug=False,
                   enable_asserts=False, num_devices=NCORES)

    xT_d = nc.dram_tensor("xT", [C, T], f32r, kind="ExternalInput").ap()
    wT_d = nc.dram_tensor("wT", [C, C], f32r, kind="ExternalInput").ap()
    out_d = nc.dram_tensor("out", [T, C], f32, kind="ExternalOutput").ap()
    if CFG_DEBUG:
        dbg_xHT = nc.dram_tensor("dbg_xHT", [C, T], f32, kind="ExternalOutput").ap()
        dbg_xH = nc.dram_tensor("dbg_xH", [P, QT * C], mybir.dt.bfloat16, kind="ExternalOutput").ap()
        dbg_wei = nc.dram_tensor("dbg_wei", [P, T], mybir.dt.bfloat16, kind="ExternalOutput").ap()
        dbg_weiT = nc.dram_tensor("dbg_weiT", [P, T], mybir.dt.bfloat16, kind="ExternalOutput").ap()

    with tile.TileContext(nc) as tc:
        with tc.tile_pool(name="pers", bufs=1) as pers, \
             tc.tile_pool(name="stats", bufs=2) as statsp, \
             tc.tile_pool(name="blk", bufs=8, space="PSUM") as ps512, \
             tc.tile_pool(name="stag", bufs=3) as stagp:

            # ---- constants
            # strict-causal additive mask for the diagonal block:
            # diagmask[p, f] = 0 if f < p else NEG_BIG
            diagmask = pers.tile([P, P], f32, tag="diagmask")
            nc.gpsimd.memset(diagmask[:], 0.0)
            nc.gpsimd.affine_select(
                out=diagmask[:], in_=diagmask[:],
                compare_op=mybir.AluOpType.is_gt,
                fill=NEG_BIG, base=0, pattern=[[-1, P]], channel_multiplier=1,
            )
            # -1 everywhere except row 0 (zeroes the fully-masked first row)
            rowmask0 = pers.tile([P, 1], f32, tag="rowmask0")
            nc.gpsimd.memset(rowmask0[:], -1.0)
            nc.gpsimd.memset(rowmask0[0:1, :], 0.0)

            # ---- persistent arrays
            # xT per c-chunk: tb0 and tb1 as separate [128,512] tiles (fine
            # arrival granularity for phase-1a's first stages), tb2+tb3 as one
            # [128,1024] tile (arrives during compute). Loads split across the
            # sync HWDGE and gpsimd SWDGE rings.
            xT_sb = [[pers.tile([P, 512], f32r, tag=f"xT{c}_0", name=f"xT{c}_0"),
                      pers.tile([P, 512], f32r, tag=f"xT{c}_1", name=f"xT{c}_1"),
                      pers.tile([P, 1024], f32r, tag=f"xT{c}_h1", name=f"xT{c}_h1")]
                     for c in range(CCH)]

            def xT_slice(c, tb, off, n):
                # columns [off, off+n) within t-block tb of c-chunk c
                if tb < 2:
                    return xT_sb[c][tb][:, off:off + n]
                base = 512 * (tb - 2)
                return xT_sb[c][2][:, base + off:base + off + n]
            xHT_sb = [pers.tile([P, T], f32r, tag=f"xHT{d}", name=f"xHT{d}") for d in range(CCH)]

            # ---- phase 1a: xHT[d, t] = W_H @ x.T  (f32r)
            # wT loaded in two 512-column halves to halve its SBUF footprint.
            # DMA issue order matters: the first matmul group needs wT(h0) and
            # xT(tb0), so those go out first.
            with tc.tile_pool(name="wt", bufs=1) as wtp:
                if CFG_WT == "full":
                    wparts = [[wtp.tile([P, C], f32r, tag=f"wT{c}",
                                        name=f"wT{c}") for c in range(CCH)]]
                    wcols, nparts, dper = 1024, 1, 8
                elif CFG_WT == "quarters":
                    # wT in 4 quarter-column pieces, ping-pong slots (2 live)
                    wparts = [[wtp.tile([P, 256], f32r, tag=f"wT{c}q{q % 2}",
                                        name=f"wT{c}q{q}") for c in range(CCH)]
                              for q in range(4)]
                    wcols, nparts, dper = 256, 4, 2
                else:
                    wparts = [[wtp.tile([P, 512], f32r, tag=f"wT{c}",
                                        name=f"wT{c}h{h}") for c in range(CCH)]
                              for h in range(2)]
                    wcols, nparts, dper = 512, 2, 4
                for c in range(CCH):
                    nc.sync.dma_start(wparts[0][c][:],
                                      wT_d[P * c:P * (c + 1), 0:wcols])
                    # first-stage pieces (tb0) lead on both rings
                    eng = nc.sync if c < 4 else nc.gpsimd
                    eng.dma_start(xT_sb[c][0][:],
                                  xT_d[P * c:P * (c + 1), 0:512])
                for c in range(CCH):
                    eng = nc.sync if c < 4 else nc.gpsimd
                    eng.dma_start(xT_sb[c][1][:],
                                  xT_d[P * c:P * (c + 1), 512:1024])
                    nc.gpsimd.dma_start(xT_sb[c][2][:],
                                        xT_d[P * c:P * (c + 1), 1024:2048])
                for q in range(1, nparts):
                    for c in range(CCH):
                        nc.sync.dma_start(
                            wparts[q][c][:],
                            wT_d[P * c:P * (c + 1), wcols * q:wcols * (q + 1)])
                for q in range(nparts):
                    for tb in range(4):
                        for dd in range(dper):
                            d = dper * q + dd
                            pmm = ps512.tile([P, 512], f32, tag="blk")
                            for c in range(CCH):
                                nc.tensor.matmul(
                                    pmm[:],
                                    wparts[q][c][:, P * dd:P * (dd + 1)],
                                    xT_slice(c, tb, 0, 512),
                                    start=(c == 0), stop=(c == CCH - 1))
                            nc.vector.tensor_copy(
                                xHT_sb[d][:, 512 * tb:512 * (tb + 1)], pmm[:])

            # q-loop pools open only after the wT pool closes, so their SBUF
            # footprint doesn't overlap wT's lifetime (capacity accounting)
            qctx = ExitStack()
            weip = qctx.enter_context(tc.tile_pool(name="wei", bufs=2))
            weitp = qctx.enter_context(tc.tile_pool(name="weiT", bufs=2))
            outsp = qctx.enter_context(tc.tile_pool(name="osb", bufs=2))

            # natural-layout xH (bf16) as one tile; [t-tile, c] at column t*C+c
            xH_all = pers.tile([P, QT * C], bf16, tag="xH_all")
            xH_v = xH_all[:].rearrange("p (t c) -> p t c", c=C)

            # ---- phase 1b: natural-layout xH (bf16) from xHT, as a burst of
            # bf16 stagings + strided DMA-transposes (after all input loads,
            # so transposes don't interleave with copies on the DMA xbar):
            # stag[d', j*128+t'] -> xH_v[t', 8g+j, 128d+d']
            for g in range(2):
                for d in range(CCH):
                    stag = stagp.tile([P, 1024], bf16, tag="stag")
                    nc.scalar.copy(stag[:], xHT_sb[d][:, 1024 * g:1024 * (g + 1)])
                    # scalar ring: keeps DmaTranspose off the copy-heavy sync
                    # ring (DMATranspose<->DMACopy mixing corrupts data on HW);
                    # in full-wT mode xHT completes only at phase-1a's end, so
                    # these can't steal load bandwidth either
                    nc.scalar.dma_start_transpose(
                        xH_v[:, 8 * g:8 * g + 8, P * d:P * (d + 1)], stag[:])

            if CFG_DEBUG:
                for d in range(CCH):
                    nc.sync.dma_start(dbg_xHT[P * d:P * (d + 1), :],
                                      xHT_sb[d][:].bitcast(f32))
                nc.sync.dma_start(dbg_xH[:], xH_all[:])

            # ---- q-tile loop (software-pipelined emission)
            def emit_scores(i):
                blks = []
                off = 0
                tb = i // 4
                qs = P * (i % 4)
                for n in _sblocks(i):
                    pmm = ps512.tile([P, 512], f32, tag="blk")
                    for c in range(CCH):
                        nc.tensor.matmul(
                            pmm[:, :n],
                            xT_slice(c, tb, qs, P),
                            xHT_sb[c][:, off:off + n],
                            start=(c == 0), stop=(c == CCH - 1))
                    blks.append((pmm, off, n))
                    off += n
                # mask the diagonal block (last 128 columns)
                pl, offl, nl = blks[-1]
                nc.vector.tensor_add(pl[:, nl - P:nl], pl[:, nl - P:nl], diagmask[:])
                return blks

            def emit_softmax_out(i, blks):
                nblk = len(blks)
                negblk = statsp.tile([P, 4], f32, tag="negblk")
                for k, (pmm, off, n) in enumerate(blks):
                    nc.vector.reduce_max(negblk[:, k:k + 1], pmm[:, :n],
                                         axis=X, negate=True)
                if nblk > 1:
                    gneg = statsp.tile([P, 1], f32, tag="gneg")
                    nc.vector.tensor_reduce(gneg[:], negblk[:, :nblk],
                                            axis=X, op=mybir.AluOpType.min)
                    gneg_ap = gneg[:]
                else:
                    gneg_ap = negblk[:, 0:1]

                sums = statsp.tile([P, 4], f32, tag="sums")
                nc.gpsimd.memset(sums[:, :nblk], 0.0)
                S = P * (i + 1)
                wei = weip.tile([P, S], bf16, tag="wei")
                for k, (pmm, off, n) in enumerate(blks):
                    nc.scalar.activation(wei[:, off:off + n], pmm[:, :n], Exp,
                                         bias=gneg_ap, scale=1.0,
                                         accum_out=sums[:, k:k + 1])
                if nblk > 1:
                    sumtot = statsp.tile([P, 1], f32, tag="sumtot")
                    nc.vector.reduce_sum(sumtot[:], sums[:, :nblk], axis=X)
                    sumtot_ap = sumtot[:]
                else:
                    sumtot_ap = sums[:, 0:1]
                recip = statsp.tile([P, 1], f32, tag="recip")
                nc.vector.reciprocal(recip[:], sumtot_ap)
                if i == 0:
                    nc.vector.tensor_mul(recip[:], recip[:], rowmask0[:])
                else:
                    nc.vector.tensor_scalar_mul(recip[:], recip[:], -1.0)

                # transpose wei -> weiT via strided DMA-xbar (in two halves so
                # the first overlaps the remaining exp blocks):
                # wei[q, 128k+s'] -> weiT3[s', k, q]
                weiT3 = weitp.tile([P, i + 1, P], bf16, tag="weiT")
                nk = i + 1
                if CFG_SPLIT_WEIT == 2 and nk >= 2:
                    h1 = (nk // 2) * P
                    nc.scalar.dma_start_transpose(
                        weiT3[:, :nk // 2, :], wei[:, :h1])
                    nc.scalar.dma_start_transpose(
                        weiT3[:, nk // 2:, :], wei[:, h1:S])
                else:
                    nc.scalar.dma_start_transpose(weiT3[:], wei[:, :S])

                # out = wei @ xH  (bf16, accumulate over s-chunks)
                if CFG_DEBUG and i == QT - 1:
                    nc.sync.dma_start(dbg_wei[:, :S], wei[:])
                    nc.sync.dma_start(dbg_weiT[:, :S],
                                      weiT3[:].rearrange("p a b -> p (a b)"))
                osb = outsp.tile([P, C], f32, tag="osb")
                if int(os.environ.get("K_OUTP_SPLIT", "0")):
                    for cb in range(CB):
                        outp = outpp.tile([P, 512], f32, tag="outp")
                        for k in range(i + 1):
                            nc.tensor.matmul(
                                outp[:],
                                weiT3[:, k, :],
                                xH_v[:, k, 512 * cb:512 * (cb + 1)],
                                start=(k == 0), stop=(k == i))
                        sl = osb[:, 512 * cb:512 * (cb + 1)]
                        if CFG_SCALE == "dve":
                            nc.vector.tensor_scalar_mul(sl, outp[:], recip[:])
                        else:
                            nc.scalar.activation(sl, outp[:], Copy, bias=0.0,
                                                 scale=recip[:])
                else:
                    # out accumulators come from the shared blk pool: adjacent
                    # tiles rotate through different banks, so out(i) never
                    # waits on scale-copy(i-1) draining a dedicated tile
                    for cb in range(CB):
                        opc = ps512.tile([P, 512], f32, tag="blk")
                        for k in range(i + 1):
                            nc.tensor.matmul(
                                opc[:],
                                weiT3[:, k, :],
                                xH_v[:, k, 512 * cb:512 * (cb + 1)],
                                start=(k == 0), stop=(k == i))
                        sl = osb[:, 512 * cb:512 * (cb + 1)]
                        if CFG_SCALE == "dve":
                            nc.vector.tensor_scalar_mul(sl, opc[:], recip[:])
                        else:
                            nc.scalar.activation(sl, opc[:], Copy, bias=0.0,
                                                 scale=recip[:])
                nc.sync.dma_start(out_d[P * i:P * (i + 1), :], osb[:])

            # q-tile order [1..15, 0]: the tiny tile 0 makes a short tail.
            # Scores emission runs two tiles ahead so PE always has ready
            # matmuls while a tile's softmax chain drains.
            order = list(range(1, QT)) + [0]
            pending = [emit_scores(order[k]) for k in range(CFG_DEPTH)]
            for idx, i in enumerate(order):
                if idx + CFG_DEPTH < QT:
                    pending.append(emit_scores(order[idx + CFG_DEPTH]))
                emit_softmax_out(i, pending.pop(0))
            qctx.close()

    nc.compile()
    return nc


def kernel(x: np.ndarray, W_H: np.ndarray) -> np.ndarray:
    from concourse import bass_utils

    x = np.asarray(x, dtype=np.float32)
    W_H = np.asarray(W_H, dtype=np.float32)
    nc = _build()
    wT = _f32r_round(W_H.T)
    in_maps = [{"xT": _f32r_round(x[b].T), "wT": wT} for b in range(B)]
    res = bass_utils.run_bass_kernel_spmd(nc, in_maps, core_ids=list(range(NCORES)))
    return np.stack([res.results[b]["out"] for b in range(B)])


if __name__ == "__main__":
    x = np.random.randn(B, T, C).astype(np.float32)
    W = (np.random.randn(C, C) / np.sqrt(C)).astype(np.float32)
    out = kernel(x, W)
    print("out", out.shape, out.dtype)


# revision 51
# speedup vs baseline: 395.8269x; 1.0238x over previous
"""EnergyHead kernel for Trainium2 (8 NeuronCores, batch-parallel).

Computes, per batch element:
    xH = x @ W_H.T
    scores = x @ xH.T  (strict lower-triangular causal mask, diag excluded)
    wei = softmax(scores); fully-masked row 0 zeroed
    out = -(wei @ xH)

Sharding: data-parallel over B=8 across 8 cores. Each core receives
xT = f32r(x[b].T) [C,T] and wT = f32r(W_H.T) [C,C]; matmul phases 1-2 run in
float32r (11-bit mantissa, full PE speed), phase 3 (attention-weighted sum,
near-one-hot weights) in bf16. End-to-end rel err vs fp32 reference ~2e-3.
"""
import sys
import os
import functools
from contextlib import ExitStack

sys.path.insert(0, "/opt/trn_rl_repo")
import numpy as np

# experiment flags (read once at import)
CFG_WT = os.environ.get("K_WT", "full")        # quarters | halves_war
CFG_SPLIT_WEIT = int(os.environ.get("K_SPLIT_WEIT", "1"))  # 1 | 2
CFG_SCALE = os.environ.get("K_SCALE", "act")       # act | dve
CFG_DEPTH = int(os.environ.get("K_DEPTH", "3"))    # scores prefetch depth
CFG_DEBUG = int(os.environ.get("K_DEBUG", "0"))

B, T, C = 8, 2048, 1024
NCORES = 8
P = 128                      # partition dim
QT = T // P                  # 16 q-tiles
CCH = C // P                 # 8 contraction chunks
CB = C // 512                # 2 column blocks of the output
NEG_BIG = -1e30


def _f32r_round(a: np.ndarray) -> np.ndarray:
    """Round fp32 to the f32r format (round-to-nearest, 11 mantissa bits)."""
    au = np.ascontiguousarray(a, dtype=np.float32).view(np.uint32)
    return ((au.astype(np.uint64) + 0x800).astype(np.uint32) & np.uint32(0xFFFFF000)).view(np.float32)


def _sblocks(i: int) -> list[int]:
    """Split S=128*(i+1) score columns into matmul N-blocks, avoiding
    blocks <256 (f32r runs 4 cycles/row below 256) except the i=0 case."""
    S = P * (i + 1)
    if S <= 512:
        return [S]
    k4, rem = divmod(S, 512)
    if rem == 0:
        return [512] * k4
    if rem == P:
        return [512] * (k4 - 1) + [384, 256]
    return [512] * k4 + [rem]


@functools.lru_cache(maxsize=1)
def _build():
    import concourse.bacc as bacc
    import concourse.tile as tile
    from concourse import mybir
    from concourse.masks import make_identity

    f32 = mybir.dt.float32
    f32r = mybir.dt.float32r
    bf16 = mybir.dt.bfloat16
    X = mybir.AxisListType.X
    Exp = mybir.ActivationFunctionType.Exp
    Copy = mybir.ActivationFunctionType.Copy

    nc = bacc.Bacc("TRN2", target_bir_lowering=False, debug=False,
                   enable_asserts=False, num_devices=NCORES)

    xT_d = nc.dram_tensor("xT", [C, T], f32r, kind="ExternalInput").ap()
    wT_d = nc.dram_tensor("wT", [C, C], f32r, kind="ExternalInput").ap()
    out_d = nc.dram_tensor("out", [T, C], f32, kind="ExternalOutput").ap()
    if CFG_DEBUG:
        dbg_xHT = nc.dram_tensor("dbg_xHT", [C, T], f32, kind="ExternalOutput").ap()
        dbg_xH = nc.dram_tensor("dbg_xH", [P, QT * C], mybir.dt.bfloat16, kind="ExternalOutput").ap()
        dbg_wei = nc.dram_tensor("dbg_wei", [P, T], mybir.dt.bfloat16, kind="ExternalOutput").ap()
        dbg_weiT = nc.dram_tensor("dbg_weiT", [P, T], mybir.dt.bfloat16, kind="ExternalOutput").ap()

    with tile.TileContext(nc) as tc:
        with tc.tile_pool(name="pers", bufs=1) as pers, \
             tc.tile_pool(name="stats", bufs=2) as statsp, \
             tc.tile_pool(name="blk", bufs=8, space="PSUM") as ps512, \
             tc.tile_pool(name="stag", bufs=3) as stagp:

            # ---- constants
            # strict-causal additive mask for the diagonal block:
            # diagmask[p, f] = 0 if f < p else NEG_BIG
            diagmask = pers.tile([P, P], f32, tag="diagmask")
            nc.gpsimd.memset(diagmask[:], 0.0)
            nc.gpsimd.affine_select(
                out=diagmask[:], in_=diagmask[:],
                compare_op=mybir.AluOpType.is_gt,
                fill=NEG_BIG, base=0, pattern=[[-1, P]], channel_multiplier=1,
            )
            # -1 everywhere except row 0 (zeroes the fully-masked first row)
            rowmask0 = pers.tile([P, 1], f32, tag="rowmask0")
            nc.gpsimd.memset(rowmask0[:], -1.0)
            nc.gpsimd.memset(rowmask0[0:1, :], 0.0)

            # ---- persistent arrays
            # xT per c-chunk: tb0 and tb1 as separate [128,512] tiles (fine
            # arrival granularity for phase-1a's first stages), tb2+tb3 as one
            # [128,1024] tile (arrives during compute). Loads split across the
            # sync HWDGE and gpsimd SWDGE rings.
            xT_sb = [[pers.tile([P, 512], f32r, tag=f"xT{c}_0", name=f"xT{c}_0"),
                      pers.tile([P, 512], f32r, tag=f"xT{c}_1", name=f"xT{c}_1"),
                      pers.tile([P, 1024], f32r, tag=f"xT{c}_h1", name=f"xT{c}_h1")]
                     for c in range(CCH)]

            def xT_slice(c, tb, off, n):
                # columns [off, off+n) within t-block tb of c-chunk c
                if tb < 2:
                    return xT_sb[c][tb][:, off:off + n]
                base = 512 * (tb - 2)
                return xT_sb[c][2][:, base + off:base + off + n]
            xHT_sb = [pers.tile([P, T], f32r, tag=f"xHT{d}", name=f"xHT{d}") for d in range(CCH)]

            # ---- phase 1a: xHT[d, t] = W_H @ x.T  (f32r)
            # wT loaded in two 512-column halves to halve its SBUF footprint.
            # DMA issue order matters: the first matmul group needs wT(h0) and
            # xT(tb0), so those go out first.
            with tc.tile_pool(name="wt", bufs=1) as wtp:
                if CFG_WT == "full":
                    wparts = [[wtp.tile([P, C], f32r, tag=f"wT{c}",
                                        name=f"wT{c}") for c in range(CCH)]]
                    wcols, nparts, dper = 1024, 1, 8
                elif CFG_WT == "quarters":
                    # wT in 4 quarter-column pieces, ping-pong slots (2 live)
                    wparts = [[wtp.tile([P, 256], f32r, tag=f"wT{c}q{q % 2}",
                                        name=f"wT{c}q{q}") for c in range(CCH)]
                              for q in range(4)]
                    wcols, nparts, dper = 256, 4, 2
                else:
                    wparts = [[wtp.tile([P, 512], f32r, tag=f"wT{c}",
                                        name=f"wT{c}h{h}") for c in range(CCH)]
                              for h in range(2)]
                    wcols, nparts, dper = 512, 2, 4
                for c in range(CCH):
                    nc.sync.dma_start(wparts[0][c][:],
                                      wT_d[P * c:P * (c + 1), 0:wcols])
                    # first-stage pieces (tb0) lead on both rings
                    eng = nc.sync if c < 4 else nc.gpsimd
                    eng.dma_start(xT_sb[c][0][:],
                                  xT_d[P * c:P * (c + 1), 0:512])
                for c in range(CCH):
                    eng = nc.sync if c < 4 else nc.gpsimd
                    eng.dma_start(xT_sb[c][1][:],
                                  xT_d[P * c:P * (c + 1), 512:1024])
                    nc.gpsimd.dma_start(xT_sb[c][2][:],
                                        xT_d[P * c:P * (c + 1), 1024:2048])
                for q in range(1, nparts):
                    for c in range(CCH):
                        nc.sync.dma_start(
                            wparts[q][c][:],
                            wT_d[P * c:P * (c + 1), wcols * q:wcols * (q + 1)])
                for q in range(nparts):
                    for tb in range(4):
                        for dd in range(dper):
                            d = dper * q + dd
                            pmm = ps512.tile([P, 512], f32, tag="blk")
                            for c in range(CCH):
                                nc.tensor.matmul(
                                    pmm[:],
                                    wparts[q][c][:, P * dd:P * (dd + 1)],
                                    xT_slice(c, tb, 0, 512),
                                    start=(c == 0), stop=(c == CCH - 1))
                            nc.vector.tensor_copy(
                                xHT_sb[d][:, 512 * tb:512 * (tb + 1)], pmm[:])

            # q-loop pools open only after the wT pool closes, so their SBUF
            # footprint doesn't overlap wT's lifetime (capacity accounting)
            qctx = ExitStack()
            weip = qctx.enter_context(tc.tile_pool(name="wei", bufs=2))
            weitp = qctx.enter_context(tc.tile_pool(name="weiT", bufs=2))
            outsp = qctx.enter_context(tc.tile_pool(name="osb", bufs=2))

            # natural-layout xH (bf16) as one tile; [t-tile, c] at column t*C+c
            xH_all = pers.tile([P, QT * C], bf16, tag="xH_all")
            xH_v = xH_all[:].rearrange("p (t c) -> p t c", c=C)

            # ---- phase 1b: natural-layout xH (bf16) from xHT, as a burst of
            # bf16 stagings + strided DMA-transposes (after all input loads,
            # so transposes don't interleave with copies on the DMA xbar):
            # stag[d', j*128+t'] -> xH_v[t', 8g+j, 128d+d']
            for g in range(2):
                for d in range(CCH):
                    stag = stagp.tile([P, 1024], bf16, tag="stag")
                    nc.scalar.copy(stag[:], xHT_sb[d][:, 1024 * g:1024 * (g + 1)])
                    # scalar ring: keeps DmaTranspose off the copy-heavy sync
                    # ring (DMATranspose<->DMACopy mixing corrupts data on HW);
                    # in full-wT mode xHT completes only at phase-1a's end, so
                    # these can't steal load bandwidth either
                    nc.scalar.dma_start_transpose(
                        xH_v[:, 8 * g:8 * g + 8, P * d:P * (d + 1)], stag[:])

            if CFG_DEBUG:
                for d in range(CCH):
                    nc.sync.dma_start(dbg_xHT[P * d:P * (d + 1), :],
                                      xHT_sb[d][:].bitcast(f32))
                nc.sync.dma_start(dbg_xH[:], xH_all[:])

            # ---- q-tile loop (software-pipelined emission)
            def emit_scores(i):
                blks = []
                off = 0
                tb = i // 4
                qs = P * (i % 4)
                for n in _sblocks(i):
                    pmm = ps512.tile([P, 512], f32, tag="blk")
                    for c in range(CCH):
                        nc.tensor.matmul(
                            pmm[:, :n],
                            xT_slice(c, tb, qs, P),
                            xHT_sb[c][:, off:off + n],
                            start=(c == 0), stop=(c == CCH - 1))
                    blks.append((pmm, off, n))
                    off += n
                # mask the diagonal block (last 128 columns)
                pl, offl, nl = blks[-1]
                nc.vector.tensor_add(pl[:, nl - P:nl], pl[:, nl - P:nl], diagmask[:])
                return blks

            def emit_softmax_out(i, blks):
                nblk = len(blks)
                negblk = statsp.tile([P, 4], f32, tag="negblk")
                for k, (pmm, off, n) in enumerate(blks):
                    nc.vector.reduce_max(negblk[:, k:k + 1], pmm[:, :n],
                                         axis=X, negate=True)
                if nblk > 1:
                    gneg = statsp.tile([P, 1], f32, tag="gneg")
                    nc.vector.tensor_reduce(gneg[:], negblk[:, :nblk],
                                            axis=X, op=mybir.AluOpType.min)
                    gneg_ap = gneg[:]
                else:
                    gneg_ap = negblk[:, 0:1]

                sums = statsp.tile([P, 4], f32, tag="sums")
                nc.gpsimd.memset(sums[:, :nblk], 0.0)
                S = P * (i + 1)
                wei = weip.tile([P, S], bf16, tag="wei")
                for k, (pmm, off, n) in enumerate(blks):
                    nc.scalar.activation(wei[:, off:off + n], pmm[:, :n], Exp,
                                         bias=gneg_ap, scale=1.0,
                                         accum_out=sums[:, k:k + 1])
                if nblk > 1:
                    sumtot = statsp.tile([P, 1], f32, tag="sumtot")
                    nc.vector.reduce_sum(sumtot[:], sums[:, :nblk], axis=X)
                    sumtot_ap = sumtot[:]
                else:
                    sumtot_ap = sums[:, 0:1]
                recip = statsp.tile([P, 1], f32, tag="recip")
                nc.vector.reciprocal(recip[:], sumtot_ap)
                if i == 0:
                    nc.vector.tensor_mul(recip[:], recip[:], rowmask0[:])
                else:
                    nc.vector.tensor_scalar_mul(recip[:], recip[:], -1.0)

                # transpose wei -> weiT via strided DMA-xbar (in two halves so
                # the first overlaps the remaining exp blocks):
                # wei[q, 128k+s'] -> weiT3[s', k, q]
                weiT3 = weitp.tile([P, i + 1, P], bf16, tag="weiT")
                nk = i + 1
                if CFG_SPLIT_WEIT == 2 and nk >= 2:
                    h1 = (nk // 2) * P
                    nc.scalar.dma_start_transpose(
                        weiT3[:, :nk // 2, :], wei[:, :h1])
                    nc.scalar.dma_start_transpose(
                        weiT3[:, nk // 2:, :], wei[:, h1:S])
                else:
                    nc.scalar.dma_start_transpose(weiT3[:], wei[:, :S])

                # out = wei @ xH  (bf16, accumulate over s-chunks)
                if CFG_DEBUG and i == QT - 1:
                    nc.sync.dma_start(dbg_wei[:, :S], wei[:])
                    nc.sync.dma_start(dbg_weiT[:, :S],
                                      weiT3[:].rearrange("p a b -> p (a b)"))
                osb = outsp.tile([P, C], f32, tag="osb")
                if int(os.environ.get("K_OUTP_SPLIT", "0")):
                    for cb in range(CB):
                        outp = outpp.tile([P, 512], f32, tag="outp")
                        for k in range(i + 1):
                            nc.tensor.matmul(
                                outp[:],
                                weiT3[:, k, :],
                                xH_v[:, k, 512 * cb:512 * (cb + 1)],
                                start=(k == 0), stop=(k == i))
                        sl = osb[:, 512 * cb:512 * (cb + 1)]
                        if CFG_SCALE == "dve":
                            nc.vector.tensor_scalar_mul(sl, outp[:], recip[:])
                        else:
                            nc.scalar.activation(sl, outp[:], Copy, bias=0.0,
                                                 scale=recip[:])
                else:
                    # out accumulators come from the shared blk pool: adjacent
                    # tiles rotate through different banks, so out(i) never
                    # waits on scale-copy(i-1) draining a dedicated tile
                    for cb in range(CB):
                        opc = ps512.tile([P, 512], f32, tag="blk")
                        for k in range(i + 1):
                            nc.tensor.matmul(
                                opc[:],
                                weiT3[:, k, :],
                                xH_v[:, k, 512 * cb:512 * (cb + 1)],
                                start=(k == 0), stop=(k == i))
                        sl = osb[:, 512 * cb:512 * (cb + 1)]
                        if CFG_SCALE == "dve":
                            nc.vector.tensor_scalar_mul(sl, opc[:], recip[:])
                        else:
                            nc.scalar.activation(sl, opc[:], Copy, bias=0.0,
                                                 scale=recip[:])
                nc.sync.dma_start(out_d[P * i:P * (i + 1), :], osb[:])

            # q-tile order [1..15, 0]: the tiny tile 0 makes a short tail.
            # Scores emission runs two tiles ahead so PE always has ready
            # matmuls while a tile's softmax chain drains.
            _tail = int(os.environ.get("K_TAIL", "1"))
            order = list(range(_tail, QT)) + list(range(_tail - 1, -1, -1))
            pending = [emit_scores(order[k]) for k in range(CFG_DEPTH)]
            for idx, i in enumerate(order):
                if idx + CFG_DEPTH < QT:
                    pending.append(emit_scores(order[idx + CFG_DEPTH]))
                emit_softmax_out(i, pending.pop(0))
            qctx.close()

    nc.compile()
    return nc


def kernel(x: np.ndarray, W_H: np.ndarray) -> np.ndarray:
    from concourse import bass_utils

    x = np.asarray(x, dtype=np.float32)
    W_H = np.asarray(W_H, dtype=np.float32)
    nc = _build()
    wT = _f32r_round(W_H.T)
    in_maps = [{"xT": _f32r_round(x[b].T), "wT": wT} for b in range(B)]
    res = bass_utils.run_bass_kernel_spmd(nc, in_maps, core_ids=list(range(NCORES)))
    return np.stack([res.results[b]["out"] for b in range(B)])


if __name__ == "__main__":
    x = np.random.randn(B, T, C).astype(np.float32)
    W = (np.random.randn(C, C) / np.sqrt(C)).astype(np.float32)
    out = kernel(x, W)
    print("out", out.shape, out.dtype)
